# revision 12
# baseline (speedup 1.0000x reference)
"""AttentionGCNLayer Trainium2 kernel.

Per-sample computation (B=8 samples -> 8 NeuronCores, data-parallel):
  identity = x @ W_it + b_it
  gcn      = relu(adj @ (x @ W_g + b_g))
  h        = LN1(identity + gcn)
  attn     = MHSA(h)  (8 heads, D=32)
  out      = LN2(h + attn)

Key layout decisions (see inline comments):
  - scoresT layout [k-tokens on partition, q-tokens on free]: softmax exp runs
    on ScalarE (the only exp engine; the critical path) reading PSUM directly,
    and attn@V uses V as the stationary operand with column-tiling.
  - Softmax denominators via ones-matmul (partition-direction sum on PE) into
    a PSUM bank whose 32-row blocks line up with the attn-out bank rows, so
    normalization is a plain elementwise multiply (no partition broadcasts).
  - Softmax skips max-subtraction: scores*scale are in [-1.3, 1.3] by
    construction (LN'd inputs, D=32), exp is safe in fp32.
  - rsqrt for LayerNorm = exp(-0.5*ln(var+eps)) so ScalarE stays on the
    natural_log_exp_and_others table set the whole kernel (no table thrash).
"""

import sys

sys.path.insert(0, "/opt/trn_rl_repo")

import numpy as np

import concourse.bass as bass
import concourse.tile as tile
from concourse import bacc, mybir
from concourse.bass_utils import run_bass_kernel_spmd
from concourse.masks import make_identity

F32 = mybir.dt.float32
BF16 = mybir.dt.bfloat16
AF = mybir.ActivationFunctionType
ALU = mybir.AluOpType

B, N, CI, CO, H, D = 8, 1024, 128, 256, 8, 32
P = 128
MT = N // P  # 8 token chunks
EPS = 1e-5
SCALE = float(1.0 / np.sqrt(np.float32(D)))
NCORES = 8


def _ln(nc, pool, src_ap, dst_ap, g_bc, b_bc, eps_sb):
    """LayerNorm over the free dim (256 channels) of a [128, 256] tile.

    src_ap may be SBUF; dst_ap is SBUF. rsqrt via exp(-0.5*ln(var+eps)) keeps
    ScalarE on the ln/exp table set.
    """
    stats = pool.tile([P, 6], F32, tag="ln_stats")
    nc.vector.bn_stats(out=stats, in_=src_ap)
    mv = pool.tile([P, 2], F32, tag="ln_mv")
    nc.vector.bn_aggr(out=mv, in_=stats)
    lnv = pool.tile([P, 1], F32, tag="ln_lnv")
    nc.scalar.activation(lnv, mv[:, 1:2], AF.Ln, bias=eps_sb)
    rstd = pool.tile([P, 1], F32, tag="ln_rstd")
    nc.scalar.activation(rstd, lnv, AF.Exp, scale=-0.5)
    cen = pool.tile([P, CO], F32, tag="ln_cen")
    # (x - mean) * rstd
    nc.vector.tensor_scalar(
        out=cen, in0=src_ap, scalar1=mv[:, 0:1], scalar2=rstd,
        op0=ALU.subtract, op1=ALU.mult,
    )
    # * gamma + beta  (gamma/beta broadcast along partitions at load time)
    nc.vector.tensor_mul(out=cen, in0=cen, in1=g_bc)
    nc.vector.tensor_add(out=dst_ap, in0=cen, in1=b_bc)


def _split_pe_waits(nc):
    """Walrus limits fused sync-wait commands per PE instruction (S3_LW has a
    single slot). Move all but one wait of each PE instruction onto preceding
    NoOps (one wait per NoOp)."""
    n_split = 0
    for fn in nc.m.functions:
        for bb in fn.blocks:
            new = []
            for inst in bb.instructions:
                si = getattr(inst, "sync_info", None)
                eng = getattr(inst, "engine", None)
                if (eng == mybir.EngineType.PE and si is not None
                        and si.on_wait and len(si.on_wait) > 1):
                    waits = list(si.on_wait)
                    for w in waits[:-1]:
                        n_split += 1
                        new.append(mybir.InstNoOp(
                            name=f"{inst.name}-wsplit{n_split}",
                            engine=inst.engine,
                            sync_info=mybir.SyncInfo(on_wait=[w], on_update=[]),
                            bass_nofuse=True,
                        ))
                    inst.sync_info = mybir.SyncInfo(
                        on_wait=waits[-1:], on_update=list(si.on_update))
                new.append(inst)
            bb.instructions = new
    return n_split


def build_bass():
    nc = bacc.Bacc()

    x_d = nc.dram_tensor("x", (N, CI), F32, kind="ExternalInput")
    adj_d = nc.dram_tensor("adj", (N, N), F32, kind="ExternalInput")
    wit_d = nc.dram_tensor("W_it", (CI, CO), F32, kind="ExternalInput")
    bit_d = nc.dram_tensor("b_it", (CO,), F32, kind="ExternalInput")
    wg_d = nc.dram_tensor("W_g", (CI, CO), F32, kind="ExternalInput")
    bg_d = nc.dram_tensor("b_g", (CO,), F32, kind="ExternalInput")
    wq_d = nc.dram_tensor("W_q", (CO, CO), F32, kind="ExternalInput")
    bq_d = nc.dram_tensor("b_q", (CO,), F32, kind="ExternalInput")
    wk_d = nc.dram_tensor("W_k", (CO, CO), F32, kind="ExternalInput")
    bk_d = nc.dram_tensor("b_k", (CO,), F32, kind="ExternalInput")
    wv_d = nc.dram_tensor("W_v", (CO, CO), F32, kind="ExternalInput")
    bv_d = nc.dram_tensor("b_v", (CO,), F32, kind="ExternalInput")
    wo_d = nc.dram_tensor("W_o", (CO, CO), F32, kind="ExternalInput")
    bo_d = nc.dram_tensor("b_o", (CO,), F32, kind="ExternalInput")
    g1_d = nc.dram_tensor("g1", (CO,), F32, kind="ExternalInput")
    be1_d = nc.dram_tensor("beta1", (CO,), F32, kind="ExternalInput")
    g2_d = nc.dram_tensor("g2", (CO,), F32, kind="ExternalInput")
    be2_d = nc.dram_tensor("beta2", (CO,), F32, kind="ExternalInput")
    out_d = nc.dram_tensor("out", (N, CO), F32, kind="ExternalOutput")

    with tile.TileContext(nc) as tc:
        from contextlib import ExitStack

        with ExitStack() as ctx:
            singles = ctx.enter_context(tc.tile_pool(name="singles", bufs=1))
            stemp = ctx.enter_context(tc.tile_pool(name="stemp", bufs=3))
            adj_pool = ctx.enter_context(tc.tile_pool(name="adj", bufs=2))
            adjT_pool = ctx.enter_context(tc.tile_pool(name="adjT", bufs=2))
            expT_pool = ctx.enter_context(tc.tile_pool(name="expT", bufs=4))
            ytile_pool = ctx.enter_context(tc.tile_pool(name="ytile", bufs=2))

            # ---------------- Phase 0: constants / weights ----------------
            ident_sb = singles.tile([P, P], F32)
            make_identity(nc, ident_sb)
            ones_sb = singles.tile([P, D], BF16)
            nc.vector.memset(ones_sb, 1.0)
            eps_sb = singles.tile([P, 1], F32)
            nc.vector.memset(eps_sb, EPS)

            wit_sb = singles.tile([P, CO], F32)
            nc.sync.dma_start(wit_sb, wit_d[:])
            wg_sb = singles.tile([P, CO], F32)
            nc.sync.dma_start(wg_sb, wg_d[:])

            def load_w2(dram):
                t = singles.tile([P, 2, CO], F32, tag=f"w2_{dram.name}")
                nc.sync.dma_start(t, dram[:].rearrange("(ko ki) n -> ki ko n", ki=P))
                return t

            wq_sb = load_w2(wq_d)
            wk_sb = load_w2(wk_d)
            wv_sb = load_w2(wv_d)
            wo_sb = load_w2(wo_d)

            def load_pp(dram):  # per-partition scalars [128, 2] (co-chunked)
                t = singles.tile([P, 2], F32, tag=f"pp_{dram.name}")
                nc.sync.dma_start(t, dram[:].rearrange("(ko ki) -> ki ko", ki=P))
                return t

            bq_sb = load_pp(bq_d)
            bk_sb = load_pp(bk_d)

            def load_bc(dram):  # broadcast along partitions: [128, 256]
                t = singles.tile([P, CO], F32, tag=f"bc_{dram.name}")
                src = dram[:]
                bcast = bass.AP(tensor=src.tensor, offset=src.offset,
                                ap=[[0, P]] + list(src.ap))
                nc.gpsimd.dma_start(out=t, in_=bcast)
                return t

            bit_bc = load_bc(bit_d)
            bg_bc = load_bc(bg_d)
            bv_bc = load_bc(bv_d)
            bo_bc = load_bc(bo_d)
            g1_bc = load_bc(g1_d)
            be1_bc = load_bc(be1_d)
            g2_bc = load_bc(g2_d)
            be2_bc = load_bc(be2_d)

            # persistent activations
            xT_sb = singles.tile([P, MT, P], F32)      # x^T  [ci, m]
            t_sb = singles.tile([P, MT, CO], F32)      # x@W_g + b_g   [tok, c]
            id_sb = singles.tile([P, MT, CO], F32)     # x@W_it + b_it [tok, c]
            h_sb = singles.tile([P, MT, CO], F32)      # LN1 out       [tok, c]
            hT_sb = singles.tile([P, 2, N], F32)       # h^T           [c, tok]
            qT_sb = singles.tile([P, 2, N], F32)       # q^T           [c, tok]
            kT_sb = singles.tile([P, 2, N], F32)       # k^T           [c, tok]
            v_sb = singles.tile([P, MT, CO], BF16)     # v             [tok, c]
            outT_sb = singles.tile([P, 2, N], F32)     # attn-out^T    [c, tok]

            with ExitStack() as pre:
                tr_ps = pre.enter_context(
                    tc.tile_pool(name="tr_ps", bufs=2, space="PSUM"))
                mm_ps = pre.enter_context(
                    tc.tile_pool(name="mm_ps", bufs=2, space="PSUM"))
                qkv_ps = pre.enter_context(
                    tc.tile_pool(name="qkv_ps", bufs=2, space="PSUM"))

                # Warm-up transpose so PE observes the gpsimd sem (identity
                # production) before the real transposes; keeps transpose-mode
                # matmuls at <=1 sync wait (walrus S3_LW sync-slot limit).
                warm_ps = tr_ps.tile([P, 4, P], F32, tag="tr")
                nc.tensor.transpose(warm_ps[:, 0, :], ident_sb, ident_sb)

                # ---------------- Phase 1: load & transpose x ----------------
                x_sb = singles.tile([P, MT, CI], F32)
                nc.sync.dma_start(x_sb, x_d[:].rearrange("(mt p) c -> p mt c", p=P))
                for half in range(2):
                    ps = tr_ps.tile([P, 4, P], F32, tag="tr")
                    for i in range(4):
                        m = half * 4 + i
                        nc.tensor.transpose(ps[:, i, :], x_sb[:, m, :], ident_sb)
                    nc.vector.tensor_copy(xT_sb[:, half * 4:half * 4 + 4, :], ps)

                # ---------------- Phase 2: t = x@W_g+b, id = x@W_it+b --------
                for m in range(MT):
                    tp = mm_ps.tile([P, CO], F32, tag="mm256")
                    nc.tensor.matmul(tp, xT_sb[:, m, :], wg_sb, start=True, stop=True)
                    nc.vector.tensor_add(t_sb[:, m, :], tp, bg_bc)
                    ip = mm_ps.tile([P, CO], F32, tag="mm256")
                    nc.tensor.matmul(ip, xT_sb[:, m, :], wit_sb, start=True, stop=True)
                    nc.vector.tensor_add(id_sb[:, m, :], ip, bit_bc)

                # ---------------- Phase 3: adj^T, gcn, LN1, h^T --------------
                adj_r = adj_d[:].rearrange("(mt p) k -> p mt k", p=P)
                for m in range(MT):
                    ab = adj_pool.tile([P, N], F32)
                    nc.sync.dma_start(ab, adj_r[:, m, :])
                    at = adjT_pool.tile([P, MT, P], F32)
                    for half in range(2):
                        ps = tr_ps.tile([P, 4, P], F32, tag="tr")
                        for i in range(4):
                            k = half * 4 + i
                            nc.tensor.transpose(
                                ps[:, i, :], ab[:, k * P:(k + 1) * P], ident_sb)
                        # psum->sbuf copies split between ScalarE and VectorE
                        if half == 0:
                            nc.scalar.copy(at[:, 0:4, :], ps)
                        else:
                            nc.vector.tensor_copy(at[:, 4:8, :], ps)
                    gp = mm_ps.tile([P, CO], F32, tag="mm256")
                    for k in range(MT):
                        nc.tensor.matmul(gp, at[:, k, :], t_sb[:, k, :],
                                         start=(k == 0), stop=(k == MT - 1))
                    # s = identity + relu(gcn)
                    s_t = stemp.tile([P, CO], F32, tag="s_t")
                    nc.vector.scalar_tensor_tensor(
                        out=s_t, in0=gp, scalar=0.0, in1=id_sb[:, m, :],
                        op0=ALU.max, op1=ALU.add)
                    _ln(nc, stemp, s_t, h_sb[:, m, :], g1_bc, be1_bc, eps_sb)
                    # h^T for this chunk (2 transposes into one psum tile)
                    ps = tr_ps.tile([P, 4, P], F32, tag="tr")
                    nc.tensor.transpose(ps[:, 0, :], h_sb[:, m, 0:P], ident_sb)
                    nc.tensor.transpose(ps[:, 1, :], h_sb[:, m, P:CO], ident_sb)
                    nc.scalar.copy(hT_sb[:, 0, m * P:(m + 1) * P], ps[:, 0, :])
                    nc.scalar.copy(hT_sb[:, 1, m * P:(m + 1) * P], ps[:, 1, :])

                # ---------------- Phase 4: q^T, k^T (c-major), v (tok-major) -
                for oc in range(2):
                    for qh in range(2):
                        qsl = slice(qh * 512, (qh + 1) * 512)
                        qp = qkv_ps.tile([P, 512], F32, tag="qkv")
                        for kc in range(2):
                            nc.tensor.matmul(
                                qp, wq_sb[:, kc, oc * P:(oc + 1) * P],
                                hT_sb[:, kc, qsl],
                                start=(kc == 0), stop=(kc == 1))
                        nc.vector.tensor_scalar_add(
                            qT_sb[:, oc, qsl], qp, bq_sb[:, oc:oc + 1])
                        kp = qkv_ps.tile([P, 512], F32, tag="qkv")
                        for kc in range(2):
                            nc.tensor.matmul(
                                kp, wk_sb[:, kc, oc * P:(oc + 1) * P],
                                hT_sb[:, kc, qsl],
                                start=(kc == 0), stop=(kc == 1))
                        nc.vector.tensor_scalar_add(
                            kT_sb[:, oc, qsl], kp, bk_sb[:, oc:oc + 1])
                for m in range(MT):
                    vp = mm_ps.tile([P, CO], F32, tag="mm256")
                    for kc in range(2):
                        nc.tensor.matmul(vp, hT_sb[:, kc, m * P:(m + 1) * P],
                                         wv_sb[:, kc, :],
                                         start=(kc == 0), stop=(kc == 1))
                    nc.vector.tensor_add(v_sb[:, m, :], vp, bv_bc)

            # ---------------- Phase 5+6: attention + output ----------------
            with ExitStack() as att:
                sc_ps = att.enter_context(
                    tc.tile_pool(name="sc_ps", bufs=2, space="PSUM"))
                acc_ps = att.enter_context(
                    tc.tile_pool(name="acc_ps", bufs=1, space="PSUM"))
                proj_ps = att.enter_context(
                    tc.tile_pool(name="proj_ps", bufs=2, space="PSUM"))

                def proj_ln2_store(m):
                    """attn-out projection + residual + LN2 + DMA for chunk m."""
                    pp = proj_ps.tile([P, CO], F32, tag="proj")
                    for cc in range(2):
                        nc.tensor.matmul(pp, outT_sb[:, cc, m * P:(m + 1) * P],
                                         wo_sb[:, cc, :],
                                         start=(cc == 0), stop=(cc == 1))
                    s2 = stemp.tile([P, CO], F32, tag="s2")
                    nc.vector.tensor_add(s2, pp, bo_bc)
                    nc.vector.tensor_add(s2, s2, h_sb[:, m, :])
                    yt = ytile_pool.tile([P, CO], F32)
                    _ln(nc, stemp, s2, yt, g2_bc, be2_bc, eps_sb)
                    nc.sync.dma_start(
                        out_d[:].rearrange("(mt p) c -> p mt c", p=P)[:, m, :], yt)

                for qh in range(2):
                    qsl = slice(qh * 512, (qh + 1) * 512)
                    for g in range(2):
                        outb = acc_ps.tile([P, 512], F32, tag="outb")
                        denb = acc_ps.tile([P, 512], F32, tag="denb")
                        for k in range(MT):
                            for tp in range(2):  # head pair within group
                                sc = sc_ps.tile([P, 1024], F32, tag="sc")
                                for j2 in range(2):
                                    hh = 4 * g + 2 * tp + j2   # global head
                                    bp = 32 * (hh % 4)
                                    nc.tensor.matmul(
                                        sc[:, j2 * 512:(j2 + 1) * 512],
                                        kT_sb[bp:bp + 32, g, k * P:(k + 1) * P],
                                        qT_sb[bp:bp + 32, g, qsl],
                                        start=True, stop=True,
                                        tile_position=(bp, 0))
                                ex = expT_pool.tile([P, 1024], BF16, tag="ex")
                                nc.scalar.activation(ex, sc, AF.Exp, scale=SCALE)
                                for j2 in range(2):
                                    hh = 4 * g + 2 * tp + j2
                                    cp = 32 * (hh % 4)
                                    esl = slice(j2 * 512, (j2 + 1) * 512)
                                    nc.tensor.matmul(
                                        outb[cp:cp + 32, :],
                                        v_sb[:, k, hh * D:(hh + 1) * D],
                                        ex[:, esl],
                                        start=(k == 0), stop=(k == MT - 1),
                                        tile_position=(0, cp),
                                        skip_group_check=True)
                                    nc.tensor.matmul(
                                        denb[cp:cp + 32, :],
                                        ones_sb, ex[:, esl],
                                        start=(k == 0), stop=(k == MT - 1),
                                        tile_position=(0, cp),
                                        skip_group_check=True)
                        rec = stemp.tile([P, 512], F32, tag="rec")
                        nc.vector.reciprocal(rec, denb)
                        nc.vector.tensor_mul(outT_sb[:, g, qsl], outb, rec)
                    # both head groups of this token half done -> drain output
                    for m in range(qh * 4, qh * 4 + 4):
                        proj_ln2_store(m)

    nc.finalize()
    return nc


_CACHE = {}


def _get_nc():
    if "nc" not in _CACHE:
        _CACHE["nc"] = build_bass()
    return _CACHE["nc"]


def run(inputs, trace=False):
    nc = _get_nc()
    shared = {k: np.ascontiguousarray(np.asarray(v, np.float32))
              for k, v in inputs.items() if k not in ("x", "adj")}
    x = np.ascontiguousarray(np.asarray(inputs["x"], np.float32))
    adj = np.ascontiguousarray(np.asarray(inputs["adj"], np.float32))
    in_maps = []
    for b in range(NCORES):
        m = dict(shared)
        m["x"] = x[b]
        m["adj"] = adj[b]
        in_maps.append(m)
    res = run_bass_kernel_spmd(nc, in_maps, core_ids=list(range(NCORES)),
                               trace=trace)
    out = np.stack([res.results[b]["out"] for b in range(NCORES)], axis=0)
    return out, res


def kernel(**inputs):
    out, _ = run(inputs, trace=False)
    return out
